# revision 62
# baseline (speedup 1.0000x reference)
"""CPGA Trainium2 Bass kernel.

Two SPMD launches over 8 NeuronCores, sharded (batch, row-half):
  stage 1: LN(low)/LN(high) -> fused -> mask logits + aligned features ->
           per-core partial class-prototype sums (streaming spatial softmax).
  host   : combine partials -> cf -> memory mix -> k/v + folded biases.
  stage 2: query conv path, cross-attention over 19 prototypes, proj+residual,
           LN, conv-FFN (1x1 -> depthwise 3x3 -> gelu -> 1x1), final residual.

Layout: channels on partitions, pixels on the free dim (all SBUF tiles are
partition-first). Depthwise 3x3 convs run on TensorE as fp8 diagonal
matmuls: per dc a DoubleRow pair packs the (dr=-1, dr=0) taps via an
overlapping strided rhs view plus a dr=+1 single -> 6 matmuls per group.
The FFN 1x1 convs are fp8 DoubleRow GEMMs (contraction 256 per matmul,
weights prescaled by 64 to stay out of fp8 subnormals). q/z activations
live in full-row contiguous fp8 buffers so the dw convs read across tile
borders with no halo copies. LayerNorm stats use 1/C-scaled ones-matmul
partition broadcasts; rstd = exp(-0.5*ln(var+eps)); apply is (x-mu)*rstd.
The stage-2 softmax chain is interleaved step-by-step with build_z/ffn
matmul groups so the in-order PE queue never stalls on ACT/DVE latency,
and ACT ops are batch-ordered by table (2 ACT_TABLE_LOADs per tile).
"""

import numpy as np
import ml_dtypes

import concourse.bass as bass
import concourse.mybir as mybir
from concourse import bacc
from concourse.tile import TileContext
from concourse.bass_utils import run_bass_kernel_spmd

# Prefer the combined Ln+Exp activation table so per-tile Ln/Exp/Square
# sequences don't thrash ACT_TABLE_LOADs (the insertion pass picks the
# first table containing each func).
from concourse import hw_specs as _hw_specs
_orig_get_act_tables = _hw_specs.get_activation_tables

def _act_tables_combined_first(arch):
    # Keep canonical order/indices (walrus maps set-id by index); advertise
    # only the two tables we want selected so the greedy pass never picks a
    # set lacking a func we'll need next (Ln+Exp+Square live together in
    # natural_log_exp_and_others; Gelu needs gelu_and_others).
    tabs = dict(_orig_get_act_tables(arch))
    keep = ("natural_log_exp_and_others", "gelu_and_others")
    return {k: (v if k in keep else type(v)()) for k, v in tabs.items()}

bacc.get_activation_tables = _act_tables_combined_first

BF = mybir.dt.bfloat16
F32 = mybir.dt.float32
F32R = mybir.dt.float32r
F8 = mybir.dt.float8e4
AL = mybir.AluOpType
AF = mybir.ActivationFunctionType
PM = mybir.MatmulPerfMode.DoubleRow
bf16 = ml_dtypes.bfloat16
f8e4 = ml_dtypes.float8_e4m3fn
WS = 64.0          # fp8 weight prescale (keeps 0.02-scale weights out of subnormals)

B, C, H, W = 4, 256, 128, 128
NCL, NH, HD = 19, 8, 32
SCALE = HD ** -0.5
MOM = 0.1
EPS = 1e-5
NCORES = 8
R = 64            # rows per core chunk
S1_T = 16         # stage-1 tiles of 512 px (64 rows)
S2_T = 17         # stage-2 tiles (68 rows incl 2-row halo each side)
TN = 512          # pixels per tile
QH = 130          # free-dim halo for dw conv tiles
NQ = S2_T * TN + 2 * QH   # full-row q/z buffer width (17 tiles + edge halos)
LN2 = float(np.log(0.5))



# ----------------------------------------------------------------------------
# stage 1 builder
# ----------------------------------------------------------------------------

def build_stage1():
    """Transpose-free stage 1: class-prototype sums via direct px-partition
    matmuls. Per 128-px block B: mk^T[px,19] = sum_ct c1a[:,ct,B]^T @ wm2;
    e^T = exp(mk^T); xa^T[px,256] = sum_ct f[:,ct,B]^T @ WalgT; then
    S += e^T.T @ xa^T and Z += e^T.T @ 1 accumulate in PSUM across all
    blocks. No TensorE transposes, no spatial-softmax accumulator reads.
    """
    nc = bacc.Bacc()
    lo = nc.dram_tensor("lo", [128, 2, S1_T * TN], BF, kind="ExternalInput")
    hi = nc.dram_tensor("hi", [128, 2, S1_T * TN], BF, kind="ExternalInput")
    ones = nc.dram_tensor("ones", [128, 128], BF, kind="ExternalInput")
    wm1 = nc.dram_tensor("wm1", [128, 2, 128], BF, kind="ExternalInput")
    wm2 = nc.dram_tensor("wm2", [128, 2, NCL], BF, kind="ExternalInput")
    walT = nc.dram_tensor("walT", [128, 2, 256], BF, kind="ExternalInput")
    S_out = nc.dram_tensor("S_out", [NCL, 256], F32, kind="ExternalOutput")
    Z_out = nc.dram_tensor("Z_out", [NCL, 1], F32, kind="ExternalOutput")

    with TileContext(nc) as tc:
        with (
            tc.tile_pool(name="cst", bufs=1) as cst,
            tc.tile_pool(name="sb", bufs=7) as sb,
            tc.tile_pool(name="st", bufs=5) as st,
            tc.tile_pool(name="ps_a", bufs=1, space="PSUM") as ps_a,
            tc.tile_pool(name="ps_b", bufs=2, space="PSUM") as ps_b,
        ):
            ones_t = cst.tile([128, 128], BF, tag="ones")
            nc.sync.dma_start(ones_t[:], ones[:])
            wm1_t = cst.tile([128, 2, 128], BF, tag="wm1")
            nc.sync.dma_start(wm1_t[:], wm1[:])
            wm2_t = cst.tile([128, 2, NCL], BF, tag="wm2")
            nc.sync.dma_start(wm2_t[:], wm2[:])
            walT_t = cst.tile([128, 2, 256], BF, tag="walT")
            nc.sync.dma_start(walT_t[:], walT[:])
            cfz = ps_a.tile([NCL, 512], F32, tag="cf")
            cf = cfz[:, 0:256]
            zps = cfz[:, 256:257]
            epsb = cst.tile([128, 1], F32, tag="epsb")
            nc.vector.memset(epsb[:], EPS)
            ln2b = cst.tile([128, 1], F32, tag="ln2b")
            nc.vector.memset(ln2b[:], LN2)

            # flat 2D bf16 tiles throughout: sliced [128,2,TN] operands run DVE
            # at 1x, flat [128,TN] at 2x (measured 601 vs 336 ns). One-deep
            # software pipeline: stats(t) || consume(t-1), so each engine's
            # FIFO only sees ops whose inputs are already ready (no DVE
            # head-of-line blocking on the Ln/Exp chain).
            state = {}

            def stats_phase(t):
                sl = slice(t * TN, (t + 1) * TN)
                x_f = {}
                for nm, src in (("lo", lo), ("hi", hi)):
                    for ct in range(2):
                        xt = sb.tile([128, TN], BF, tag=f"{nm}{ct}", name="xt")
                        nc.sync.dma_start(xt[:], src[:, ct, sl])
                        x_f[(nm, ct)] = xt
                sq_f = {}
                for ct in range(2):
                    sql_ = sb.tile([128, TN], BF, tag=f"sql{ct}", name="sql_")
                    nc.gpsimd.tensor_tensor(sql_[:], x_f[("lo", ct)][:],
                                            x_f[("lo", ct)][:], op=AL.mult)
                    sq_f[("lo", ct)] = sql_
                    sqh_ = sb.tile([128, TN], BF, tag=f"sqh{ct}", name="sqh_")
                    nc.gpsimd.tensor_tensor(sqh_[:], x_f[("hi", ct)][:],
                                            x_f[("hi", ct)][:], op=AL.mult)
                    sq_f[("hi", ct)] = sqh_

                def ln_stats(nm, tag):
                    s1 = ps_a.tile([128, TN], F32, tag="s1", bufs=2)
                    nc.tensor.matmul(s1[:], ones_t[:], x_f[(nm, 0)][:], start=True, stop=False)
                    nc.tensor.matmul(s1[:], ones_t[:], x_f[(nm, 1)][:], start=False, stop=True)
                    s2 = ps_a.tile([128, TN], F32, tag="s2", bufs=2)
                    nc.tensor.matmul(s2[:], ones_t[:], sq_f[(nm, 0)][:], start=True, stop=False)
                    nc.tensor.matmul(s2[:], ones_t[:], sq_f[(nm, 1)][:], start=False, stop=True)
                    mu2 = st.tile([128, TN], F32, tag="mu2" + tag)
                    nc.scalar.activation(mu2[:], s1[:], AF.Square, scale=1.0 / C)
                    for ct in range(2):
                        ctr = sb.tile([128, TN], BF, tag=f"ctr{nm}{ct}", name="ctr")
                        nc.vector.scalar_tensor_tensor(ctr[:], s1[:], -1.0 / C,
                                                       x_f[(nm, ct)][:],
                                                       op0=AL.mult, op1=AL.add)
                        x_f[(nm, ct)] = ctr
                    var = st.tile([128, TN], F32, tag="var" + tag)
                    nc.vector.scalar_tensor_tensor(var[:], s2[:], 1.0 / C, mu2[:],
                                                   op0=AL.mult, op1=AL.subtract)
                    nc.scalar.activation(var[:], var[:], AF.Ln, bias=epsb[:])
                    r2 = st.tile([128, TN], BF, tag="r2" + tag)   # 0.5 * rstd
                    nc.scalar.activation(r2[:], var[:], AF.Exp, scale=-0.5, bias=ln2b[:])
                    return r2

                rl2 = ln_stats("lo", "l")
                rh2 = ln_stats("hi", "h")
                state[t] = (x_f, rl2, rh2)

            def consume_phase(t):
                x_f, rl2, rh2 = state.pop(t)
                # fused = (lo - mu_l)*rl2 + (hi - mu_h)*rh2, per flat ct tile
                f_f = {}
                for ct in range(2):
                    t1 = sb.tile([128, TN], BF, tag=f"t1{ct}", name="t1")
                    nc.gpsimd.tensor_tensor(t1[:], x_f[("lo", ct)][:], rl2[:], op=AL.mult)
                    t2 = sb.tile([128, TN], BF, tag=f"t2{ct}", name="t2")
                    nc.vector.tensor_tensor(t2[:], x_f[("hi", ct)][:], rh2[:], op=AL.mult)
                    f_c = sb.tile([128, TN], BF, tag=f"f{ct}", name="f_c")
                    nc.vector.tensor_tensor(f_c[:], t1[:], t2[:], op=AL.add)
                    f_f[ct] = f_c

                # mask conv1 (block-diag grouped)
                c1a = {}
                for ct in range(2):
                    c1p = ps_b.tile([128, TN], F32, tag="mmx", name="c1p", bufs=3)
                    nc.tensor.matmul(c1p[:], wm1_t[:, ct, :], f_f[ct][:], start=True, stop=True)
                    c1c = sb.tile([128, TN], BF, tag=f"c1a{ct}", name="c1c")
                    if ct == 0:
                        nc.scalar.copy(c1c[:], c1p[:])
                    else:
                        nc.vector.tensor_copy(c1c[:], c1p[:])
                    c1a[ct] = c1c

                # per 128-px block: mk^T -> e^T; xa^T; accumulate S and Z
                for blk in range(4):
                    bsl = slice(blk * 128, (blk + 1) * 128)
                    mkp = ps_b.tile([128, NCL], F32, tag="mmx", name="mkp", bufs=3)
                    for ct in range(2):
                        nc.tensor.matmul(mkp[:], c1a[ct][:, bsl], wm2_t[:, ct, :],
                                         start=(ct == 0), stop=(ct == 1))
                    eT = sb.tile([128, NCL], BF, tag="eT")
                    nc.scalar.activation(eT[:], mkp[:], AF.Exp)
                    xap = ps_b.tile([128, 256], F32, tag="mmx", name="xap", bufs=3)
                    for ct in range(2):
                        nc.tensor.matmul(xap[:], f_f[ct][:, bsl], walT_t[:, ct, :],
                                         start=(ct == 0), stop=(ct == 1))
                    xaTs = sb.tile([128, 256], BF, tag="xaTs")
                    if blk == 3:
                        nc.vector.tensor_copy(xaTs[:], xap[:])
                    else:
                        nc.scalar.copy(xaTs[:], xap[:])
                    first = (t == 0 and blk == 0)
                    last = (t == S1_T - 1 and blk == 3)
                    nc.tensor.matmul(cfz[:, 0:256], eT[:], xaTs[:], start=first, stop=last)
                    nc.tensor.matmul(cfz[:, 256:257], eT[:], ones_t[:, 0:1], start=first, stop=last)

            for t in range(S1_T):
                stats_phase(t)
                consume_phase(t)

            S_sb = cst.tile([NCL, 256], F32, tag="S_sb")
            nc.vector.tensor_copy(S_sb[:], cfz[:, 0:256])
            nc.sync.dma_start(S_out[:], S_sb[:])
            z_sb = cst.tile([NCL, 1], F32, tag="z_sb")
            nc.vector.tensor_copy(z_sb[:], cfz[:, 256:257])
            nc.sync.dma_start(Z_out[:], z_sb[:])

    nc.finalize()
    return nc


# ----------------------------------------------------------------------------
# stage 2 builder
# ----------------------------------------------------------------------------

# ----------------------------------------------------------------------------
# stage 2 builder
# ----------------------------------------------------------------------------

def build_stage2():
    """Fully fused stage 2: one software-pipelined loop per tile.

    iter t: build_z(t-4) | attn(t-2) | ffn(t-5) | build_query(t). The
    attention softmax chain (lp -> exp -> zsum -> recip -> bcast -> mult)
    is interleaved step-by-step with build_z / ffn dw-conv groups so the
    in-order PE queue always has independent matmul work while ACT/DVE
    latency drains. ACT ops are batch-ordered by table (LN+softmax exps
    first, then the gelu block) -> 2 ACT_TABLE_LOADs per iteration. The
    ones matrix is 1/C-scaled so LN stats come out of PSUM as mu/E[x2]
    directly; the softmax reciprocal is cast to bf16 before the
    partition-broadcast matmul. The query 1x1 conv is folded into K
    (kbd = kq) and the output projection into V (pvbd), so q/av
    intermediates never materialize.
    """
    nc = bacc.Bacc()
    NPX = S2_T * TN
    lo16 = nc.dram_tensor("lo16", [128, 2, NPX], BF, kind="ExternalInput")
    ones = nc.dram_tensor("ones", [128, 128], BF, kind="ExternalInput")
    kbd = nc.dram_tensor("kbd", [128, 2, 152], BF, kind="ExternalInput")
    pvbd = nc.dram_tensor("pvbd", [128, 2, 256], BF, kind="ExternalInput")
    bsum = nc.dram_tensor("bsum", [76, 76], BF, kind="ExternalInput")
    bexp = nc.dram_tensor("bexp", [128, 2], F32, kind="ExternalInput")
    # depthwise weights as fp8 diagonal blocks: _p holds (dr=-1, dr=0) pairs
    # per dc for DoubleRow, _s the dr=+1 singles; all prescaled by WS.
    wqdw_p = nc.dram_tensor("wqdw_p", [128, 2, 3, 2, 128], F8, kind="ExternalInput")
    wqdw_s = nc.dram_tensor("wqdw_s", [128, 2, 3, 128], F8, kind="ExternalInput")
    wmlp1 = nc.dram_tensor("wmlp1", [128, 2, 1024], F8, kind="ExternalInput")
    wdwm_p = nc.dram_tensor("wdwm_p", [128, 8, 3, 2, 128], F8, kind="ExternalInput")
    wdwm_s = nc.dram_tensor("wdwm_s", [128, 8, 3, 128], F8, kind="ExternalInput")
    wmlp2 = nc.dram_tensor("wmlp2", [128, 8, 256], F8, kind="ExternalInput")
    bprj = nc.dram_tensor("bprj", [128, 2], F32, kind="ExternalInput")
    b1 = nc.dram_tensor("b1", [128, 8], F32, kind="ExternalInput")
    bdw = nc.dram_tensor("bdw", [128, 8], F32, kind="ExternalInput")
    b2 = nc.dram_tensor("b2", [128, 2], F32, kind="ExternalInput")
    zmask = nc.dram_tensor("zmask", [128, 2, TN], F8, kind="ExternalInput")
    OUT = nc.dram_tensor("OUT", [128, 2, S1_T * TN], F32, kind="ExternalOutput")

    def pair_view(src2d, off_a, delta, n):
        """[128, 2, n] DoubleRow rhs view V[c,i,p] = src2d[c, off_a+i*delta+p]."""
        u = src2d[:, off_a:off_a + n].unsqueeze(1).broadcast_to((128, 2, n))
        v = u.ap
        v[1] = [delta, 2]
        c = u.copy()
        c.ap = v
        return c

    def pair_view4(src2d, off_a, delta, w, nr, nw):
        """4D paired-block view V[c,i,r,j] = src2d[c, off_a+i*delta+r*w+j]."""
        base = src2d[:, off_a:off_a + nr * w].rearrange("p (r w) -> p r w", w=w)
        u = base[:, :, 0:nw].unsqueeze(1).broadcast_to((128, 2, nr, nw))
        v = u.ap
        v[1] = [delta, 2]
        c = u.copy()
        c.ap = v
        return c

    with TileContext(nc) as tc:
        with (
            tc.tile_pool(name="cst", bufs=1) as cst,
            tc.tile_pool(name="lop", bufs=5) as lop,
            tc.tile_pool(name="sb2", bufs=3) as sb2,
            tc.tile_pool(name="outp", bufs=5) as outp,
            tc.tile_pool(name="ylp", bufs=3) as ylp,
            tc.tile_pool(name="ps", bufs=2, space="PSUM") as ps,
        ):
            ones_t = cst.tile([128, 128], BF, tag="ones"); nc.sync.dma_start(ones_t[:], ones[:])
            kbd_t = cst.tile([128, 2, 152], BF, tag="kbd"); nc.sync.dma_start(kbd_t[:], kbd[:])
            pvbd_t = cst.tile([128, 2, 256], BF, tag="pvbd"); nc.sync.dma_start(pvbd_t[:], pvbd[:])
            bsum_t = cst.tile([76, 76], BF, tag="bsum"); nc.sync.dma_start(bsum_t[:], bsum[:])
            bexp_t = cst.tile([128, 2], F32, tag="bexp"); nc.sync.dma_start(bexp_t[:], bexp[:])
            wqdwp_t = cst.tile([128, 2, 3, 2, 128], F8, tag="wqdwp"); nc.sync.dma_start(wqdwp_t[:], wqdw_p[:])
            wqdws_t = cst.tile([128, 2, 3, 128], F8, tag="wqdws"); nc.sync.dma_start(wqdws_t[:], wqdw_s[:])
            wmlp1_t = cst.tile([128, 2, 1024], F8, tag="wmlp1"); nc.sync.dma_start(wmlp1_t[:], wmlp1[:])
            wdwmp_t = cst.tile([128, 8, 3, 2, 128], F8, tag="wdwmp"); nc.sync.dma_start(wdwmp_t[:], wdwm_p[:])
            wdwms_t = cst.tile([128, 8, 3, 128], F8, tag="wdwms"); nc.sync.dma_start(wdwms_t[:], wdwm_s[:])
            wmlp2_t = cst.tile([128, 8, 256], F8, tag="wmlp2"); nc.sync.dma_start(wmlp2_t[:], wmlp2[:])
            bprj_t = cst.tile([128, 2], F32, tag="bprj"); nc.sync.dma_start(bprj_t[:], bprj[:])
            b1_t = cst.tile([128, 8], F32, tag="b1"); nc.sync.dma_start(b1_t[:], b1[:])
            bdw_t = cst.tile([128, 8], F32, tag="bdw"); nc.sync.dma_start(bdw_t[:], bdw[:])
            b2_t = cst.tile([128, 2], F32, tag="b2"); nc.sync.dma_start(b2_t[:], b2[:])
            zm_t = cst.tile([128, 2, TN], F8, tag="zm"); nc.sync.dma_start(zm_t[:], zmask[:])
            epsb = cst.tile([128, 1], F32, tag="epsb")
            nc.vector.memset(epsb[:], EPS)

            # full-row contiguous q/z buffers (fp8): neighbouring tiles are
            # physically adjacent, so the dw convs read across tile borders
            # directly and no halo copies are needed. Edges zeroed once.
            qbig = cst.tile([128, 2, NQ], F8, tag="qbig")
            nc.vector.memset(qbig[:, :, 0:QH], 0.0)
            nc.vector.memset(qbig[:, :, QH + S2_T * TN:], 0.0)
            zbig = cst.tile([128, 8, NQ], F8, tag="zbig")
            nc.vector.memset(zbig[:, :, 0:QH], 0.0)
            nc.vector.memset(zbig[:, :, QH + S2_T * TN:], 0.0)

            lo_a = {}
            out_a = {}
            yl_a = {}
            e_a = {}
            rz_a = {}
            rzb_a = {}
            f01_a = {}
            qd_a = {}
            en_a = {}
            sq_a = {}
            rlm_a = {}

            # per-tile LN stats -> (rstd bf16 tile, mu psum tile). ones is
            # 1/C-scaled, so s1 = mu and s2 = E[x^2] directly; the apply step
            # computes (x - s1) * rstd, so mu*rstd never materializes.
            def ln_tile(x0, x1, nm, sq=None):
                if sq is None:
                    sq = sb2.tile([128, 2, TN], BF, tag="sq", name="sq" + nm, bufs=3)
                    nc.gpsimd.tensor_tensor(sq[:, 0, :], x0, x0, op=AL.mult)
                    nc.gpsimd.tensor_tensor(sq[:, 1, :], x1, x1, op=AL.mult)
                s1 = ps.tile([128, TN], F32, tag="st", name="s1" + nm)
                nc.tensor.matmul(s1[:], ones_t[:], x0, start=True, stop=False)
                nc.tensor.matmul(s1[:], ones_t[:], x1, start=False, stop=True)
                s2 = ps.tile([128, TN], F32, tag="st", name="s2" + nm)
                nc.tensor.matmul(s2[:], ones_t[:], sq[:, 0, :], start=True, stop=False)
                nc.tensor.matmul(s2[:], ones_t[:], sq[:, 1, :], start=False, stop=True)
                mu2 = sb2.tile([128, TN], F32, tag="mu2", name="mu2" + nm)
                nc.scalar.activation(mu2[:], s1[:], AF.Square)
                var = sb2.tile([128, TN], F32, tag="var", name="var" + nm)
                nc.vector.tensor_tensor(var[:], s2[:], mu2[:], op=AL.subtract)
                nc.scalar.activation(var[:], var[:], AF.Ln, bias=epsb[:])
                rl = sb2.tile([128, TN], BF, tag="rl", name="rl" + nm)
                nc.scalar.activation(rl[:], var[:], AF.Exp, scale=-0.5)
                return rl, s1

            def bq_dma(t):
                sl = slice(t * TN, (t + 1) * TN)
                lo_t = lop.tile([128, 2, TN], BF, tag="lo", name="lo_t")
                nc.sync.dma_start(lo_t[:], lo16[:, :, sl])
                lo_a[t] = lo_t

            def bq_sq(t):
                lo_t = lo_a[t]
                sq = sb2.tile([128, 2, TN], BF, tag="sq", name="sqq", bufs=3)
                nc.gpsimd.tensor_tensor(sq[:, 0, :], lo_t[:, 0, :], lo_t[:, 0, :], op=AL.mult)
                nc.gpsimd.tensor_tensor(sq[:, 1, :], lo_t[:, 1, :], lo_t[:, 1, :], op=AL.mult)
                sq_a[t] = sq

            def bq_stats(t):
                lo_t = lo_a[t]
                rlm_a[t] = ln_tile(lo_t[:, 0, :], lo_t[:, 1, :], "q", sq=sq_a.pop(t))

            def bq_apply(t):
                lo_t = lo_a[t]
                rl, s1 = rlm_a.pop(t)
                ctr = sb2.tile([128, 2, TN], BF, tag="lnap", name="ctrq", bufs=2)
                for ct in range(2):
                    nc.vector.tensor_tensor(ctr[:, ct, :], lo_t[:, ct, :], s1[:],
                                            op=AL.subtract)
                for ct in range(2):
                    nc.gpsimd.tensor_tensor(qbig[:, ct, QH + t * TN:QH + (t + 1) * TN],
                                            ctr[:, ct, :], rl[:], op=AL.mult)

            def dw9p(psum, wp, ws_, src):
                """fp8 depthwise 3x3: per dc a DoubleRow pair (dr=-1,0) plus a
                dr=+1 single; 6 matmuls instead of 9. The dc=0 pair runs first
                full-width so start=True clears the whole psum tile."""
                out3 = psum[:].rearrange("p (r w) -> p r w", w=128)
                nc.tensor.matmul(psum[:], wp[:, 1], pair_view(src, QH - 128, 128, TN),
                                 start=True, stop=False, perf_mode=PM)
                nc.tensor.matmul(psum[:], ws_[:, 1], src[:, QH + 128:QH + 128 + TN],
                                 start=False, stop=False)
                # dc = -1: out col 128r+1+j <- src col off0 + 128r + j
                rhs3m = src[:, QH + 128:QH + 128 + TN].rearrange(
                    "p (r w) -> p r w", w=128)
                nc.tensor.matmul(out3[:, :, 1:128], wp[:, 0],
                                 pair_view4(src, QH - 128, 128, 128, 4, 127),
                                 start=False, stop=False, perf_mode=PM)
                nc.tensor.matmul(out3[:, :, 1:128], ws_[:, 0], rhs3m[:, :, 0:127],
                                 start=False, stop=False)
                # dc = +1: out col 128r+j <- src col off0 + 128r + j + 1
                nc.tensor.matmul(out3[:, :, 0:127], wp[:, 2],
                                 pair_view4(src, QH - 127, 128, 128, 4, 127),
                                 start=False, stop=False, perf_mode=PM)
                nc.tensor.matmul(out3[:, :, 0:127], ws_[:, 2], rhs3m[:, :, 1:128],
                                 start=False, stop=True)

            def attn_qdw(s, ct):
                if ct == 0:
                    qd = sb2.tile([128, 2, TN], BF, tag="qd")
                    qd_a[s] = qd
                qd = qd_a[s]
                qdp = ps.tile([128, TN], F32, tag="mm", name="qdp", bufs=3)
                dw9p(qdp, wqdwp_t[:, ct], wqdws_t[:, ct],
                     qbig[:, ct, s * TN:s * TN + 2 * QH + TN])
                # copy on ACT: keeps the DVE FIFO clear ahead of the softmax
                # reciprocals (Copy is in every act table)
                nc.scalar.copy(qd[:, ct, :], qdp[:])

            def qk2a(s, hf):
                qd = qd_a[s]
                if hf == 0:
                    e_a[s] = []
                    rz_a[s] = sb2.tile([76, 2, TN], F32, tag="rz", bufs=2, name="rz")
                lp = ps.tile([128, TN], F32, tag="at", name="lp", bufs=1)
                for kt in range(2):
                    nc.tensor.matmul(lp[0:76, :], kbd_t[:, kt, hf * 76:hf * 76 + 76],
                                     qd[:, kt, :], start=(kt == 0), stop=(kt == 1))
                e_h = sb2.tile([76, TN], BF, tag="eh%d" % hf)
                nc.scalar.activation(e_h[:], lp[0:76, :], AF.Exp, scale=-SCALE,
                                     bias=bexp_t[0:76, hf:hf + 1])
                e_a[s].append(e_h)
                if hf == 1:
                    del qd_a[s]

            def qk2b(s, hf):
                # Zb[m,p] = sum_n [quarter(m)==quarter(n)] e[n,p]: block-ones
                # matmul broadcasts the per-quarter softmax sum to all 76 rows
                zb = ps.tile([76, TN], F32, tag="at", name="zb", bufs=1)
                nc.tensor.matmul(zb[:], bsum_t[:], e_a[s][hf][:],
                                 start=True, stop=True)
                nc.vector.reciprocal_approx_fast(rz_a[s][:, hf, :], zb[:])

            def en_mult(s, hf):
                en = sb2.tile([76, TN], BF, tag="en%d" % hf, name="en")
                nc.vector.tensor_tensor(en[:], e_a[s][hf][:], rz_a[s][:, hf, :],
                                        op=AL.mult)
                en_a.setdefault(s, []).append(en)
                if hf == 1:
                    del e_a[s]
                    del rz_a[s]

            def attn_proj(s, mt):
                en = en_a[s]
                if mt == 0:
                    out_t = outp.tile([128, 2, TN], BF, tag="out")
                    out_a[s] = out_t
                out_t = out_a[s]
                op_ = ps.tile([128, TN], F32, tag="mm", name="op_", bufs=3)
                for hf in range(2):
                    nc.tensor.matmul(op_[:], pvbd_t[0:76, hf, mt * 128:(mt + 1) * 128],
                                     en[hf][:], start=(hf == 0), stop=(hf == 1))
                nc.vector.scalar_tensor_tensor(out_t[:, mt, :], op_[:],
                                               bprj_t[:, mt:mt + 1],
                                               lo_a[s][:, mt, :], op0=AL.add, op1=AL.add)
                if mt == 1:
                    del en_a[s]
                    del lo_a[s]

            sqo_a = {}

            def attn_ln_sq(s, ct):
                # out squares issued eagerly right after each attn_proj half so
                # the GPSIMD work drains before attn_ln's stats matmuls issue
                if ct == 0:
                    sqo_a[s] = sb2.tile([128, 2, TN], BF, tag="sq", name="sqo",
                                        bufs=3)
                nc.gpsimd.tensor_tensor(sqo_a[s][:, ct, :], out_a[s][:, ct, :],
                                        out_a[s][:, ct, :], op=AL.mult)

            def attn_ln(s):
                out_t = out_a[s]
                ro, s1o = ln_tile(out_t[:, 0, :], out_t[:, 1, :], "o",
                                  sq=sqo_a.pop(s))
                yl_t = ylp.tile([128, 2, TN], F8, tag="yl")
                yl_a[s] = yl_t
                ctr = sb2.tile([128, 2, TN], BF, tag="lnap", name="ctro", bufs=2)
                for ct in range(2):
                    nc.vector.tensor_tensor(ctr[:, ct, :], out_t[:, ct, :], s1o[:],
                                            op=AL.subtract)
                for ct in range(2):
                    nc.gpsimd.tensor_tensor(yl_t[:, ct, :], ctr[:, ct, :], ro[:],
                                            op=AL.mult)

            def build_z_g(t, g):
                yl_t = yl_a[t]
                zs = zbig[:, g, QH + t * TN:QH + (t + 1) * TN]
                m1p = ps.tile([128, TN], F32, tag="mm", name="m1p", bufs=3)
                nc.tensor.matmul(m1p[:], wmlp1_t[:, :, g * 128:(g + 1) * 128],
                                 yl_t[:], start=True, stop=True, perf_mode=PM)
                if g % 2 == 0:
                    nc.scalar.activation(zs, m1p[:], AF.Identity,
                                         bias=b1_t[:, g:g + 1])
                else:
                    nc.vector.tensor_scalar(zs, m1p[:],
                                            b1_t[:, g:g + 1], None, op0=AL.add)
                if t == 0:
                    nc.vector.tensor_tensor(zs, zs, zm_t[:, 0, :], op=AL.mult)
                elif t == S2_T - 1:
                    nc.vector.tensor_tensor(zs, zs, zm_t[:, 1, :], op=AL.mult)

            def build_z_fin(t):
                del yl_a[t]

            gel2_a = {}

            def ffn_g(s, g):
                if g == 0:
                    f0 = ps.tile([128, TN], F32, tag="f01", name="f0")
                    f1 = ps.tile([128, TN], F32, tag="f01", name="f1")
                    f01_a[s] = (f0, f1)
                f0, f1 = f01_a[s]
                dwp = ps.tile([128, TN], F32, tag="mm", name="dwp", bufs=3)
                dw9p(dwp, wdwmp_t[:, g], wdwms_t[:, g],
                     zbig[:, g, s * TN:s * TN + 2 * QH + TN])
                # psum carries WS^2 * dw(z); gelu rescales and adds the true bias
                if g % 2 == 0:
                    gel2 = sb2.tile([128, 2, TN], F8, tag="gel", bufs=2)
                    gel2_a[s] = gel2
                gel2 = gel2_a[s]
                nc.scalar.activation(gel2[:, g % 2, :], dwp[:], AF.Gelu,
                                     scale=1.0 / (WS * WS), bias=bdw_t[:, g:g + 1])
                if g % 2 == 1:
                    k = g // 2
                    nc.tensor.matmul(f0[:], wmlp2_t[:, 2 * k:2 * k + 2, 0:128],
                                     gel2[:], start=(k == 0), stop=(k == 3),
                                     perf_mode=PM)
                    nc.tensor.matmul(f1[:], wmlp2_t[:, 2 * k:2 * k + 2, 128:256],
                                     gel2[:], start=(k == 0), stop=(k == 3),
                                     perf_mode=PM)

            def ffn_fin(s):
                f0, f1 = f01_a.pop(s)
                del gel2_a[s]
                if s == 0:
                    px0, px1, o0 = 256, TN, 0
                elif s == S2_T - 1:
                    px0, px1, o0 = 0, 256, (S2_T - 1) * TN - 256
                else:
                    px0, px1, o0 = 0, TN, s * TN - 256
                n = px1 - px0
                for ct, fps in enumerate((f0, f1)):
                    fin = sb2.tile([128, TN], F32, tag="fin", name="fin")
                    nc.vector.tensor_scalar(fin[:, 0:n], fps[:, px0:px1],
                                            1.0 / WS, b2_t[:, ct:ct + 1],
                                            op0=AL.mult, op1=AL.add)
                    nc.vector.tensor_tensor(fin[:, 0:n], fin[:, 0:n],
                                            out_a[s][:, ct, px0:px1], op=AL.add)
                    nc.sync.dma_start(OUT[:, ct, o0:o0 + n], fin[:, 0:n])
                del out_a[s]

            # software pipeline: attn(t-2) | z(t-4) | ffn(t-5) | query(t).
            # The softmax chain steps (qk2a -> qk2b -> en_mm -> en_mult ->
            # proj) are spaced out with build_z groups and ffn dw groups so
            # the in-order PE queue has independent matmuls while each
            # chain link's ACT/DVE producer drains. ACT order per iter:
            # [bq Ln/Exp, attn Exp x2] (exp table), [gelu x8] (gelu table),
            # [attn_ln Ln/Exp] (exp table) -> 2 table loads.
            for t in range(S2_T + 5):
                bz = 4 <= t < S2_T + 4      # build_z(t-4)
                qk = 2 <= t < S2_T + 2      # attn(t-2)
                fn = 5 <= t < S2_T + 5      # ffn(t-5)
                if t == 0:
                    bq_dma(0)
                if t + 1 < S2_T:
                    bq_dma(t + 1)
                if t < S2_T:
                    bq_sq(t)
                if bz:
                    build_z_g(t - 4, 0)
                    build_z_g(t - 4, 1)
                if qk:
                    attn_qdw(t - 2, 0)
                if bz:
                    build_z_g(t - 4, 2)
                    build_z_g(t - 4, 3)
                if qk:
                    attn_qdw(t - 2, 1)
                if t < S2_T:
                    bq_stats(t)
                if qk:
                    qk2a(t - 2, 0)
                if bz:
                    build_z_g(t - 4, 4)
                    build_z_g(t - 4, 5)
                if qk:
                    qk2a(t - 2, 1)
                if bz:
                    build_z_g(t - 4, 6)
                    build_z_g(t - 4, 7)
                if qk:
                    qk2b(t - 2, 0)
                if fn:
                    ffn_g(t - 5, 0)
                if qk:
                    qk2b(t - 2, 1)
                if fn:
                    ffn_g(t - 5, 1)
                if qk:
                    en_mult(t - 2, 0)
                if fn:
                    ffn_g(t - 5, 2)
                if qk:
                    en_mult(t - 2, 1)
                if fn:
                    ffn_g(t - 5, 3)
                    ffn_g(t - 5, 4)
                if qk:
                    attn_proj(t - 2, 0)
                    attn_ln_sq(t - 2, 0)
                if fn:
                    ffn_g(t - 5, 5)
                if qk:
                    attn_proj(t - 2, 1)
                    attn_ln_sq(t - 2, 1)
                if fn:
                    ffn_g(t - 5, 6)
                    ffn_g(t - 5, 7)
                if t < S2_T:
                    bq_apply(t)
                if fn:
                    ffn_fin(t - 5)
                if qk:
                    attn_ln(t - 2)
                if bz:
                    build_z_fin(t - 4)

    nc.finalize()
    return nc


# ----------------------------------------------------------------------------
# host packing
# ----------------------------------------------------------------------------

def _chunk(x, b, r0, r1, pad_lo, pad_hi):
    """x[b] rows [r0-pad_lo, r1+pad_hi) zero-clamped -> [128, 2, n*128]."""
    lo_pad = np.zeros((C, pad_lo, W), np.float32)
    hi_pad = np.zeros((C, pad_hi, W), np.float32)
    lo_src = x[b, :, max(r0 - pad_lo, 0):r0, :]
    if lo_src.shape[1] > 0:
        lo_pad[:, pad_lo - lo_src.shape[1]:, :] = lo_src
    hi_src = x[b, :, r1:min(r1 + pad_hi, H), :]
    if hi_src.shape[1] > 0:
        hi_pad[:, :hi_src.shape[1], :] = hi_src
    full = np.concatenate([lo_pad, np.asarray(x[b, :, r0:r1, :], np.float32), hi_pad], axis=1)
    n = full.shape[1]
    return np.ascontiguousarray(full.reshape(2, 128, n * W).transpose(1, 0, 2))


def _bcast_rowsel():
    m = np.zeros((128, 4 * 128), np.float32)
    for i, r in enumerate((0, 32, 64, 96)):
        m[r, i * 128:(i + 1) * 128] = 1.0
    return m.astype(bf16)


_S1 = None
_S2 = None
_last_s1_inputs = None
_last_s2_inputs = None


def kernel(**inp):
    global _S1, _S2
    f32 = np.float32
    low = np.asarray(inp["low"], f32)
    high = np.asarray(inp["high"], f32)
    g_low = np.asarray(inp["g_low"], f32); b_low = np.asarray(inp["b_low"], f32)
    g_high = np.asarray(inp["g_high"], f32); b_high = np.asarray(inp["b_high"], f32)
    g_mlp = np.asarray(inp["g_mlp"], f32); b_mlp = np.asarray(inp["b_mlp"], f32)
    w_q_dw = np.asarray(inp["w_q_dw"], f32); b_q_dw = np.asarray(inp["b_q_dw"], f32)
    w_q_pw = np.asarray(inp["w_q_pw"], f32)[:, :, 0, 0]; b_q_pw = np.asarray(inp["b_q_pw"], f32)
    w_ml1 = np.asarray(inp["w_ml1"], f32)[:, :, 0, 0]
    w_ml2 = np.asarray(inp["w_ml2"], f32)[:, :, 0, 0]
    w_align = np.asarray(inp["w_align"], f32)[:, :, 0, 0]
    w_kv = np.asarray(inp["w_kv"], f32); b_kv = np.asarray(inp["b_kv"], f32)
    memory = np.asarray(inp["memory"], f32)
    w_proj = np.asarray(inp["w_proj"], f32)[:, :, 0, 0]; b_proj = np.asarray(inp["b_proj"], f32)
    w_mlp1 = np.asarray(inp["w_mlp1"], f32)[:, :, 0, 0]; b_mlp1 = np.asarray(inp["b_mlp1"], f32)
    w_mlp_dw = np.asarray(inp["w_mlp_dw"], f32); b_mlp_dw = np.asarray(inp["b_mlp_dw"], f32)
    w_mlp2 = np.asarray(inp["w_mlp2"], f32)[:, :, 0, 0]; b_mlp2 = np.asarray(inp["b_mlp2"], f32)

    assert np.allclose(g_low, g_high), "kernel requires g_low == g_high"

    def dense_grouped(wg, groups):
        o, ipg = wg.shape
        d = np.zeros((o, ipg * groups), f32)
        opg = o // groups
        for g in range(groups):
            d[g * opg:(g + 1) * opg, g * ipg:(g + 1) * ipg] = wg[g * opg:(g + 1) * opg]
        return d

    Wm1 = dense_grouped(w_ml1, 4)
    Wal = dense_grouped(w_align, 4)
    Wm1g = Wm1 * g_low[None, :]
    Walg = Wal * g_low[None, :]
    bb = (b_low + b_high) * 0.5
    xa_bias = Wal @ bb
    ones128 = np.ones((128, 128), f32)
    ident = np.eye(128, dtype=f32)

    def pf(x):  # [k, ...] stacked lhsT tiles -> partition-first
        return np.ascontiguousarray(np.moveaxis(x, 1, 0)) if False else x

    wm1_h = np.ascontiguousarray(
        np.stack([Wm1g.T[0:128, 0:128], Wm1g.T[128:256, 128:256]]).transpose(1, 0, 2)).astype(bf16)
    wm2_h = np.ascontiguousarray(
        np.stack([w_ml2.T[0:128], w_ml2.T[128:256]]).transpose(1, 0, 2)).astype(bf16)
    # walT is the rhs layout for the px-partition xa^T matmul:
    # walT[p, ct, c'] = Walg[c', ct*128 + p]
    walT_h = np.ascontiguousarray(
        Walg.T.reshape(2, 128, 256).transpose(1, 0, 2)).astype(bf16)

    s1_core = []
    for core in range(NCORES):
        b, hf = core // 2, core % 2
        r0 = hf * R
        s1_core.append(dict(
            lo=_chunk(low, b, r0, r0 + R, 0, 0).astype(bf16),
            hi=_chunk(high, b, r0, r0 + R, 0, 0).astype(bf16),
            ones=ones128.astype(bf16),
            wm1=wm1_h, wm2=wm2_h, walT=walT_h,
        ))

    global _last_s1_inputs
    _last_s1_inputs = s1_core
    if _S1 is None:
        _S1 = build_stage1()
    res1 = run_bass_kernel_spmd(_S1, s1_core, core_ids=list(range(NCORES)))

    S = np.zeros((B, NCL, 256), f32)
    Z = np.zeros((B, NCL), f32)
    for core in range(NCORES):
        b = core // 2
        S[b] += res1.results[core]["S_out"]
        Z[b] += res1.results[core]["Z_out"][:, 0]
    cf = S / Z[:, :, None] + xa_bias[None, None, :]
    cf = (1.0 - MOM) * cf + MOM * memory
    kv = cf @ w_kv.T + b_kv
    k, v = kv[:, :, :256], kv[:, :, 256:]

    # folded q-path biases -> per (b, head, class) logit bias
    cb1 = b_low * w_q_dw[:, 0].sum(axis=(1, 2)) + b_q_dw
    cb2 = w_q_pw @ cb1 + b_q_pw
    lbh = np.zeros((B, NH, NCL), f32)
    for h in range(NH):
        lbh[:, h, :] = np.einsum("bnd,d->bn", k[:, :, 32 * h:32 * h + 32],
                                 cb2[32 * h:32 * h + 32])

    wdw_g = w_q_dw[:, 0] * g_low[:, None, None]
    W1g = w_mlp1 * g_mlp[None, :]
    b1v = b_mlp1 + w_mlp1 @ b_mlp

    def lhsT_tiles(Wt, nk, scale=1.0, dt=bf16):
        # W [out, in] -> [128, nk, out] partition-first lhsT
        st = np.stack([Wt.T[i * 128:(i + 1) * 128] for i in range(nk)])
        return np.ascontiguousarray(st.transpose(1, 0, 2) * scale).astype(dt)

    def dw_pairs(wg, ngr):
        # wg [ngr*128, 3, 3] -> fp8 diag pair/single lhsT blocks, WS-prescaled:
        # pairs [128, ngr, 3(dc), 2(dr=-1,0), 128], singles [128, ngr, 3, 128]
        wp = np.zeros((ngr, 3, 2, 128, 128), f32)
        wsg = np.zeros((ngr, 3, 128, 128), f32)
        for g in range(ngr):
            blk = wg[g * 128:(g + 1) * 128]
            for d in range(3):           # dc = d - 1 -> kw = d
                np.fill_diagonal(wp[g, d, 0], WS * blk[:, 0, d])   # dr=-1
                np.fill_diagonal(wp[g, d, 1], WS * blk[:, 1, d])   # dr=0
                np.fill_diagonal(wsg[g, d], WS * blk[:, 2, d])     # dr=+1
        wp_h = np.ascontiguousarray(wp.transpose(3, 0, 1, 2, 4)).astype(f8e4)
        ws_h = np.ascontiguousarray(wsg.transpose(2, 0, 1, 3)).astype(f8e4)
        return wp_h, ws_h

    wmlp1_h = lhsT_tiles(W1g, 2, scale=WS, dt=f8e4)
    wmlp2_h = lhsT_tiles(w_mlp2, 8, scale=WS, dt=f8e4)
    wqdwp_h, wqdws_h = dw_pairs(wdw_g, 2)
    wdwmp_h, wdwms_h = dw_pairs(w_mlp_dw[:, 0], 8)

    s2_core = []
    for core in range(NCORES):
        b, hf = core // 2, core % 2
        r0 = hf * R
        lo_ch = _chunk(low, b, r0, r0 + R, 2, 2)
        # kbd carries kq = k_h @ Wq_pw[32h:32h+32, :] (query 1x1 folded into K);
        # pvbd carries pv = v_h @ Wproj[:, 32h:32h+32]^T (proj folded into V).
        kbd = np.zeros((2, 128, 152), f32)
        pvbd = np.zeros((2, 128, 256), f32)
        bsum = np.zeros((76, 76), f32)
        bexp = np.zeros((128, 2), f32)
        for h in range(NH):
            hf2 = h // 4
            base = (h % 4) * NCL
            j = hf2 * 76 + base
            d0 = 32 * h
            kq = k[b, :, d0:d0 + 32] @ w_q_pw[d0:d0 + 32, :]        # [19, 256]
            pv = v[b, :, d0:d0 + 32] @ w_proj[:, d0:d0 + 32].T     # [19, 256]
            for n in range(NCL):
                kbd[0, :, j + n] = kq[n, 0:128]
                kbd[1, :, j + n] = kq[n, 128:256]
                pvbd[hf2, base + n, :] = pv[n, :]
            bsum[base:base + NCL, base:base + NCL] = 1.0
            bexp[base:base + NCL, hf2] = -SCALE * lbh[b, h, :]

        zmask = np.ones((128, 2, TN), f32)
        if hf == 0:
            zmask[:, 0, 0:256] = 0.0      # tile 0: image rows -2, -1
        else:
            zmask[:, 1, 256:512] = 0.0    # tile 16: image rows 128, 129

        s2_core.append(dict(
            lo16=lo_ch.astype(bf16),
            ones=(ones128 / C).astype(bf16),
            # q dw output carries a WS factor (fp8 weights prescaled), so the
            # folded K weights absorb 1/WS
            kbd=np.ascontiguousarray(kbd.transpose(1, 0, 2) / WS).astype(bf16),
            pvbd=np.ascontiguousarray(pvbd.transpose(1, 0, 2)).astype(bf16),
            bsum=bsum.astype(bf16), bexp=bexp.astype(f32),
            wqdw_p=wqdwp_h, wqdw_s=wqdws_h,
            wmlp1=wmlp1_h, wdwm_p=wdwmp_h, wdwm_s=wdwms_h, wmlp2=wmlp2_h,
            bprj=np.ascontiguousarray(b_proj.reshape(2, 128).T).astype(f32),
            b1=np.ascontiguousarray(b1v.reshape(8, 128).T * WS).astype(f32),
            bdw=np.ascontiguousarray(b_mlp_dw.reshape(8, 128).T).astype(f32),
            b2=np.ascontiguousarray(b_mlp2.reshape(2, 128).T).astype(f32),
            zmask=zmask.astype(f8e4),
        ))

    global _last_s2_inputs
    _last_s2_inputs = s2_core
    if _S2 is None:
        _S2 = build_stage2()
    res2 = run_bass_kernel_spmd(_S2, s2_core, core_ids=list(range(NCORES)))

    out = np.zeros((B, C, H, W), np.float32)
    for core in range(NCORES):
        b, hf = core // 2, core % 2
        o = res2.results[core]["OUT"]            # [128, 2, 8192]
        o = o.transpose(1, 0, 2).reshape(C, R, W)
        out[b, :, hf * R:(hf + 1) * R, :] = o
    return out

